# revision 6
# baseline (speedup 1.0000x reference)
"""MoE MLP (B=4, T=1024, d=1024, E=8, top-2) — expert-parallel Trainium2 kernel.

Strategy (expert-parallel, per the sharding hint):
  - Router (softmax over E=8 logits, top-2, renormalize) is ~0.05% of the
    FLOPs; it runs on host as part of the token-dispatch/sharding step.
  - Token dispatch: tokens routed to expert e are gathered and sent to core e
    together with that expert's w1/w2 (cast to bf16 on host). Capacity is the
    max per-expert count padded to 128.
  - Each of the 8 cores runs the expert MLP  y = gelu(x @ w1) @ w2  in bf16
    with fp32 PSUM accumulation (>99.9% of the FLOPs).
  - Un-dispatch: host scatter-adds the two weighted expert outputs per token.

Device layout per core (all SBUF tiles are [128 partitions, free]):
  xT   [d=1024, C]    bf16   tokens on free dim, d on partitions (8 k-tiles)
  w1   [d=1024, 4096] bf16   stationary operand of matmul 1
  w2   [4096, 1024]   bf16   stationary operand of matmul 2 (32 k-tiles)
  hT   [4096, NT]     bf16   gelu(x@w1)^T, produced/consumed per token block
  yT   [1024, C]      fp32   output, DMA'd back per (d-block, token-block)
"""

import numpy as np
import ml_dtypes

D = 1024
F = 4096
E = 8
TOPK = 2
AUX_COEFF = 0.01
P = 128


def _blocks_for(C):
    """Token blocks: 512-wide (PSUM bank limit) plus one 128-multiple tail."""
    blocks = [512] * (C // 512)
    if C % 512:
        blocks.append(C % 512)
    return blocks


def _build_nc(C, blocks):
    import concourse.bass as bass
    import concourse.tile as tile
    from concourse import mybir
    from concourse.mybir import ActivationFunctionType as AF

    nc = bass.Bass("TRN2", debug=False, target_bir_lowering=False)

    xt_d = nc.dram_tensor("xt", [D, C], mybir.dt.bfloat16, kind="ExternalInput").ap()
    w1_d = nc.dram_tensor("w1", [D, F], mybir.dt.bfloat16, kind="ExternalInput").ap()
    w2_d = nc.dram_tensor("w2", [F, D], mybir.dt.bfloat16, kind="ExternalInput").ap()
    yt_d = nc.dram_tensor("yt", [D, C], mybir.dt.float32, kind="ExternalOutput").ap()

    KD = D // P   # 8  k-tiles, matmul 1 contraction over d
    NF = F // P   # 32 f-blocks of H
    ND = D // P   # 8  d-blocks of Y

    with tile.TileContext(nc) as tc:
        with (
            tc.tile_pool(name="wpool", bufs=1) as wpool,
            tc.tile_pool(name="xpool", bufs=1) as xpool,
            tc.tile_pool(name="hpool", bufs=1) as hpool,
            tc.tile_pool(name="ypool", bufs=4) as ypool,
            tc.tile_pool(name="ps1", bufs=3, space="PSUM") as ps1,
            tc.tile_pool(name="ps2", bufs=3, space="PSUM") as ps2,
        ):
            w1_sb = wpool.tile([P, KD * F], mybir.dt.bfloat16, name="w1_sb")
            w2_sb = wpool.tile([P, NF * D], mybir.dt.bfloat16, name="w2_sb")
            xt_sb = xpool.tile([P, KD * C], mybir.dt.bfloat16, name="xt_sb")

            for k in range(KD):
                nc.sync.dma_start(xt_sb[:, k * C:(k + 1) * C],
                                  xt_d[k * P:(k + 1) * P, :])
            for k in range(KD):
                nc.sync.dma_start(w1_sb[:, k * F:(k + 1) * F],
                                  w1_d[k * P:(k + 1) * P, :])
            for k in range(NF):
                nc.sync.dma_start(w2_sb[:, k * D:(k + 1) * D],
                                  w2_d[k * P:(k + 1) * P, :])

            off = 0
            for NT in blocks:
                h_sb = hpool.tile([P, NF * NT], mybir.dt.bfloat16,
                                  name="h_sb", tag="h")
                # H^T[f_block] = (w1[:, f_block].T @ xT_block), then gelu
                for f in range(NF):
                    ph = ps1.tile([P, NT], mybir.dt.float32, name="ph", tag="ph")
                    for k in range(KD):
                        nc.tensor.matmul(
                            ph[:, :],
                            w1_sb[:, k * F + f * P: k * F + (f + 1) * P],
                            xt_sb[:, k * C + off: k * C + off + NT],
                            start=(k == 0), stop=(k == KD - 1),
                        )
                    nc.scalar.activation(h_sb[:, f * NT:(f + 1) * NT], ph[:, :],
                                         AF.Gelu)
                # Y^T[d_block] = w2[:, d_block].T @ H^T
                for db in range(ND):
                    py = ps2.tile([P, NT], mybir.dt.float32, name="py", tag="py")
                    for k in range(NF):
                        nc.tensor.matmul(
                            py[:, :],
                            w2_sb[:, k * D + db * P: k * D + (db + 1) * P],
                            h_sb[:, k * NT:(k + 1) * NT],
                            start=(k == 0), stop=(k == NF - 1),
                        )
                    y_sb = ypool.tile([P, NT], mybir.dt.float32,
                                      name="y_sb", tag="y")
                    nc.vector.tensor_copy(y_sb[:, :], py[:, :])
                    nc.sync.dma_start(yt_d[db * P:(db + 1) * P, off:off + NT],
                                      y_sb[:, :])
                off += NT
    # TRN2 allows at most one semaphore wait per instruction; split the
    # multi-wait instructions Tile emits into event-semaphore chains
    # (same pass Bacc runs under target_bir_lowering).
    import bass_rust
    bass_rust.generate_event_semaphores(nc)
    return nc


def _route(xf, w_gate):
    """Top-2 softmax router + aux loss, numpy fp32 (matches the jax reference
    up to fp32 rounding; ties have measure zero for continuous inputs)."""
    N = xf.shape[0]
    logits = xf @ w_gate                       # [N, E]
    m = logits.max(axis=-1, keepdims=True)
    ex = np.exp(logits - m)
    probs = ex / ex.sum(axis=-1, keepdims=True)

    rows = np.arange(N)
    i1 = probs.argmax(axis=-1)
    p1 = probs[rows, i1]
    masked = probs.copy()
    masked[rows, i1] = -np.inf
    i2 = masked.argmax(axis=-1)
    p2 = probs[rows, i2]
    s = p1 + p2
    c1 = p1 / s
    c2 = p2 / s

    counts = np.bincount(np.concatenate([i1, i2]), minlength=E).astype(np.float32)
    f = counts / np.float32(N * TOPK)
    pmean = probs.mean(axis=0)
    aux = np.float32(AUX_COEFF) * np.float32(E) * np.float32((f * pmean).sum())
    return i1, i2, c1.astype(np.float32), c2.astype(np.float32), aux


def kernel(x, w_gate, w1, w2):
    B, T, d = x.shape
    N = B * T
    xf = np.ascontiguousarray(np.asarray(x, dtype=np.float32).reshape(N, d))
    w_gate = np.asarray(w_gate, dtype=np.float32)

    i1, i2, c1, c2, aux = _route(xf, w_gate)

    # Per-expert token index lists (each token appears under its 2 experts).
    sel = np.concatenate([i1, i2])                       # [2N]
    tok = np.concatenate([np.arange(N), np.arange(N)])   # [2N]
    cw = np.concatenate([c1, c2])                        # [2N]
    order = np.argsort(sel, kind="stable")
    sel, tok, cw = sel[order], tok[order], cw[order]
    counts = np.bincount(sel, minlength=E)
    starts = np.concatenate([[0], np.cumsum(counts)])

    C = int(-(-counts.max() // P) * P)                   # pad to 128
    blocks = _blocks_for(C)
    nc = _build_nc(C, blocks)

    w1b = np.asarray(w1, dtype=ml_dtypes.bfloat16)       # [E, d, 4d]
    w2b = np.asarray(w2, dtype=ml_dtypes.bfloat16)       # [E, 4d, d]

    in_maps = []
    idx_e, cw_e = [], []
    for e in range(E):
        te = tok[starts[e]:starts[e + 1]]
        idx_e.append(te)
        cw_e.append(cw[starts[e]:starts[e + 1]])
        xg = np.zeros((C, d), dtype=np.float32)
        xg[:len(te)] = xf[te]
        in_maps.append({
            "xt": np.ascontiguousarray(xg.T).astype(ml_dtypes.bfloat16),
            "w1": np.ascontiguousarray(w1b[e]),
            "w2": np.ascontiguousarray(w2b[e]),
        })

    from concourse.bass_utils import run_bass_kernel_spmd
    res = run_bass_kernel_spmd(nc, in_maps, core_ids=list(range(E)))

    out = np.zeros((N, d), dtype=np.float32)
    for e in range(E):
        yt = res.results[e]["yt"]                        # [d, C] fp32
        ye = yt.T[:len(idx_e[e])]                        # [c_e, d]
        out[idx_e[e]] += cw_e[e][:, None] * ye
    return out.reshape(B, T, d), aux
